# revision 1
# baseline (speedup 1.0000x reference)
"""ConvCaps dynamic-routing kernel for 8 TRN2 NeuronCores.

Strategy (data-parallel over batch B=8, one batch element per core):
  - Grouped 3x3 conv (groups=D=32) done as one matmul per group per
    pixel-tile: stationary = im2col patches [72, npx], moving = weights
    [72, 512], PSUM out [npx, 512] -> u tile in SBUF laid out
    [px_partition, D, c, d].  No u traffic to DRAM at all.
  - 3 dynamic-routing iterations run on the Vector engine entirely
    in SBUF with px on partitions: softmax over d, s/a einsums as
    multiply + segmented tensor_reduce over D (resp. c).
  - Output s [px, (c,d)] is PE-transposed to [(c,d), px] and DMA'd out.
"""

import numpy as np
from contextlib import ExitStack

import concourse.bacc as bacc
import concourse.bass as bass
import concourse.tile as tile
import concourse.mybir as mybir
from concourse.bass_utils import run_bass_kernel_spmd
from concourse.masks import make_identity

F32 = mybir.dt.float32
AF = mybir.ActivationFunctionType

B = 8
C_IN, D_IN = 8, 32
C_OUT, D_OUT = 16, 32
KS = 3
H = W = 32
HO = WO = 30
NPX = HO * WO                 # 900 output pixels per batch element
KDIM = C_IN * KS * KS         # 72 = contraction dim of the conv matmul
CD = C_OUT * D_OUT            # 512 out-channels per group
ITERS = 3
P = 128
EPS = 1e-8
# pixel tiles = groups of output rows (30 px each); partition dim <= 128
ROW_TILES = [(0, 4), (4, 4), (8, 4), (12, 4), (16, 4), (20, 4), (24, 4), (28, 2)]
DCH = 8                       # D-chunk size for the einsum passes
NCH = D_IN // DCH


def _body(ctx, tc, xb, wt, b0, out, zero_prior):
    nc = tc.nc
    consts = ctx.enter_context(tc.tile_pool(name="consts", bufs=1))
    wpool = ctx.enter_context(tc.tile_pool(name="wpool", bufs=1))
    x9pool = ctx.enter_context(tc.tile_pool(name="x9pool", bufs=1))
    upool = ctx.enter_context(tc.tile_pool(name="upool", bufs=1))
    rpool = ctx.enter_context(tc.tile_pool(name="rpool", bufs=1))
    tmppool = ctx.enter_context(tc.tile_pool(name="tmppool", bufs=2))
    opool = ctx.enter_context(tc.tile_pool(name="opool", bufs=2))
    psum_c = ctx.enter_context(tc.tile_pool(name="psum_c", bufs=6, space="PSUM"))
    psum_t = ctx.enter_context(tc.tile_pool(name="psum_t", bufs=2, space="PSUM"))

    w_sb = wpool.tile([KDIM, D_IN * CD], F32)
    nc.sync.dma_start(w_sb[:], wt)
    ident = consts.tile([P, P], F32)
    make_identity(nc, ident)
    b0_sb = consts.tile([P, D_IN, D_OUT], F32)
    nc.sync.dma_start(b0_sb[:], b0)

    for (r0, nr) in ROW_TILES:
        npx = nr * WO
        pxs = slice(0, npx)

        # ---- im2col: 9 shifted window loads; partition k = (kh*3+kw)*8 + C
        x9 = x9pool.tile([KDIM, D_IN, 4, WO], F32, tag="x9")
        for kh in range(KS):
            for kw in range(KS):
                kk = kh * KS + kw
                for j in range(nr):
                    # per-row copy keeps both DMA access patterns <= 3 dims
                    nc.sync.dma_start(
                        x9[kk * C_IN:(kk + 1) * C_IN, :, j, :],
                        xb[:, :, r0 + kh + j, kw:kw + WO],
                    )

        # ---- grouped conv: one matmul per group, psum -> u_t on ScalarE
        # u_t layout (D, c, d); strided reduces measure faster than dense
        u_t = upool.tile([P, D_IN, C_OUT, D_OUT], F32, tag="u")
        for g in range(D_IN):
            pu = psum_c.tile([P, CD], F32, tag="pu")
            nc.tensor.matmul(
                pu[pxs, :],
                x9[:, g, 0:nr, :],
                w_sb[:, g * CD:(g + 1) * CD],
                start=True, stop=True,
            )
            nc.scalar.copy(u_t[pxs, g], pu[pxs, :])

        # ---- routing state tiles
        b_t = rpool.tile([P, D_IN, D_OUT], F32, tag="b")
        c_t = rpool.tile([P, D_IN, D_OUT], F32, tag="c")
        s_t = rpool.tile([P, C_OUT, D_OUT], F32, tag="s")
        sk_t = rpool.tile([P, C_OUT, D_OUT], F32, tag="sk")
        sq_t = rpool.tile([P, C_OUT, D_OUT], F32, tag="sq")
        v_t = rpool.tile([P, C_OUT, D_OUT], F32, tag="v")
        ak_t = rpool.tile([P, DCH, D_OUT], F32, tag="ak")
        n2_t = rpool.tile([P, D_OUT], F32, tag="n2")
        r_t = rpool.tile([P, D_OUT], F32, tag="r")
        f_t = rpool.tile([P, D_OUT], F32, tag="f")
        ssum = rpool.tile([P, D_IN], F32, tag="ssum")

        nc.scalar.copy(b_t[pxs], b0_sb[pxs])

        for it in range(ITERS):
            first = it == 0
            last = it == ITERS - 1
            uniform0 = first and zero_prior

            # softmax over d (no max-subtraction: logits are O(1) here)
            if not uniform0:
                nc.scalar.activation(c_t[pxs], b_t[pxs], AF.Exp)
                nc.vector.reduce_sum(ssum[pxs], c_t[pxs],
                                     axis=mybir.AxisListType.X)
                nc.vector.reciprocal(ssum[pxs], ssum[pxs])
                nc.vector.tensor_mul(
                    c_t[pxs], c_t[pxs],
                    ssum[pxs].unsqueeze(2).broadcast_to((npx, D_IN, D_OUT)))

            # s[c,d] = sum_D c[D,d] * u[D,c,d]   (chunked over D;
            # multiplies on GpSimd, segmented reduces on Vector)
            if uniform0:
                # c is uniform 1/32: one big reduce over all of D
                red_in = u_t[pxs].rearrange("p a b c -> p (b c) a")
                nc.vector.reduce_sum(s_t[pxs], red_in,
                                     axis=mybir.AxisListType.X)
                nc.vector.tensor_scalar_mul(s_t[pxs], s_t[pxs], 1.0 / D_IN)
            else:
                for k in range(NCH):
                    dk = slice(k * DCH, (k + 1) * DCH)
                    dst = s_t if k == 0 else sk_t
                    tmp = tmppool.tile([P, DCH, C_OUT, D_OUT], F32, tag="tmp")
                    nc.gpsimd.tensor_mul(
                        tmp[pxs], u_t[pxs, dk],
                        c_t[pxs, dk].unsqueeze(2)
                        .broadcast_to((npx, DCH, C_OUT, D_OUT)))
                    red_in = tmp[pxs].rearrange("p a b c -> p (b c) a")
                    nc.vector.reduce_sum(dst[pxs], red_in,
                                         axis=mybir.AxisListType.X)
                    if k > 0:
                        nc.vector.tensor_add(s_t[pxs], s_t[pxs], sk_t[pxs])

            if last:
                break

            # squash over c: v = s * n2 / ((1+n2) * sqrt(n2+eps))
            nc.scalar.square(sq_t[pxs], s_t[pxs])
            nc.vector.reduce_sum(n2_t[pxs], sq_t[pxs].transpose([0, 2, 1]),
                                 axis=mybir.AxisListType.X)
            nc.vector.tensor_scalar_add(r_t[pxs], n2_t[pxs], EPS)
            nc.scalar.activation(r_t[pxs], r_t[pxs], AF.Sqrt)
            nc.vector.tensor_scalar_add(f_t[pxs], n2_t[pxs], 1.0)
            nc.vector.tensor_mul(f_t[pxs], f_t[pxs], r_t[pxs])
            nc.vector.reciprocal(f_t[pxs], f_t[pxs])
            nc.vector.tensor_mul(f_t[pxs], f_t[pxs], n2_t[pxs])
            nc.vector.tensor_mul(
                v_t[pxs], s_t[pxs],
                f_t[pxs].unsqueeze(1).broadcast_to((npx, C_OUT, D_OUT)))

            # b[D,d] += sum_c u[D,c,d] * v[c,d]   (chunked over D)
            for k in range(NCH):
                dk = slice(k * DCH, (k + 1) * DCH)
                tmp = tmppool.tile([P, DCH, C_OUT, D_OUT], F32, tag="tmp")
                nc.gpsimd.tensor_mul(
                    tmp[pxs], u_t[pxs, dk],
                    v_t[pxs].unsqueeze(1)
                    .broadcast_to((npx, DCH, C_OUT, D_OUT)))
                nc.vector.reduce_sum(ak_t[pxs],
                                     tmp[pxs].transpose([0, 1, 3, 2]),
                                     axis=mybir.AxisListType.X)
                nc.vector.tensor_add(b_t[pxs, dk], b_t[pxs, dk], ak_t[pxs])

        # ---- write s out as [(c,d), px]: PE transpose in 128-row blocks
        s_flat = s_t[:].rearrange("p a b -> p (a b)")
        for blk in range(CD // P):
            pt = psum_t.tile([P, 120], F32, tag="pt")
            nc.tensor.transpose(
                pt[:, pxs], s_flat[pxs, blk * P:(blk + 1) * P],
                ident[pxs, pxs])
            ob = opool.tile([P, 120], F32, tag="ob")
            nc.scalar.copy(ob[:, pxs], pt[:, pxs])
            nc.sync.dma_start(
                out[blk * P:(blk + 1) * P, r0 * WO:r0 * WO + npx],
                ob[:, pxs])


_CACHE = {}


def _build(zero_prior: bool):
    key = ("v3", zero_prior)
    if key in _CACHE:
        return _CACHE[key]
    nc = bacc.Bacc("TRN2", target_bir_lowering=False, debug=False,
                   enable_asserts=True, num_devices=B)
    xb = nc.dram_tensor("xb", [C_IN, D_IN, H, W], F32,
                        kind="ExternalInput").ap()
    wt = nc.dram_tensor("wt", [KDIM, D_IN * CD], F32,
                        kind="ExternalInput").ap()
    b0 = nc.dram_tensor("b0", [P, D_IN, D_OUT], F32,
                        kind="ExternalInput").ap()
    out = nc.dram_tensor("out", [CD, NPX], F32, kind="ExternalOutput").ap()
    with tile.TileContext(nc) as tc:
        with ExitStack() as ctx:
            _body(ctx, tc, xb, wt, b0, out, zero_prior)
    nc.compile()
    _CACHE[key] = nc
    return nc


def _prep_inputs(x, conv_w, prior):
    # weights: rows (D,c,d) x (C,kh,kw) -> [k=(kh,kw,C), (D,c,d)]
    wt = conv_w.reshape(D_IN, C_OUT, D_OUT, C_IN, KS, KS)
    wt = np.ascontiguousarray(wt.transpose(4, 5, 3, 0, 1, 2)).reshape(KDIM, D_IN * CD)
    pb = np.broadcast_to(prior.reshape(D_IN, D_OUT), (P, D_IN, D_OUT))
    b0 = np.ascontiguousarray(pb).astype(np.float32)
    in_maps = [
        {"xb": np.ascontiguousarray(x[b]), "wt": wt, "b0": b0}
        for b in range(B)
    ]
    return in_maps


def kernel(x, conv_w, prior):
    x = np.asarray(x, dtype=np.float32)
    conv_w = np.asarray(conv_w, dtype=np.float32)
    prior = np.asarray(prior, dtype=np.float32)
    zero_prior = not np.any(prior)
    nc = _build(zero_prior)
    in_maps = _prep_inputs(x, conv_w, prior)
    res = run_bass_kernel_spmd(nc, in_maps, list(range(B)))
    outs = [res.results[b]["out"].reshape(C_OUT, D_OUT, HO, WO)
            for b in range(B)]
    return np.stack(outs, axis=0).astype(np.float32)



# revision 3
# speedup vs baseline: 2.6650x; 2.6650x over previous
"""ConvCaps dynamic-routing kernel for 8 TRN2 NeuronCores.

Strategy (data-parallel over batch B=8, one batch element per core):
  - im2col done on HOST -> x9h [72, D, 900] bf16; one DMA per pixel tile.
  - Grouped 3x3 conv as one bf16 matmul per group per pixel tile:
    stationary = im2col patches [72, npx], moving = weights [72, 512],
    PSUM out [npx, 512] -> u tile in SBUF [px, D, c, d] bf16 (Scalar copy).
  - A second, PSUM-accumulating matmul chain over the 32 groups yields
    sum_D u for free -> iteration-1 s under a zero prior (uniform c).
  - Routing runs on the Vector engine in bf16 (2x packed mode):
    multiplies with 0-stride broadcast APs; reductions as contiguous
    halving add-trees (bf16 tensor_add at 2x beats 1x tensor_reduce).
  - All Scalar activations stay in ONE table set
    (natural_log_exp_and_others: Exp/Ln/Square/Copy); 1/sqrt(n2+eps)
    is computed as Exp(-0.5*Ln(n2+eps)) to avoid sqrt-set thrash.
  - Output s is DMA'd out as [px, (c,d)]; host transposes.
"""

import numpy as np
from contextlib import ExitStack

import ml_dtypes

import concourse.bacc as bacc
import concourse.tile as tile
import concourse.mybir as mybir
from concourse.bass_utils import run_bass_kernel_spmd

F32 = mybir.dt.float32
BF16 = mybir.dt.bfloat16
AF = mybir.ActivationFunctionType

B = 8
C_IN, D_IN = 8, 32
C_OUT, D_OUT = 16, 32
KS = 3
H = W = 32
HO = WO = 30
NPX = HO * WO                 # 900 output pixels per batch element
KDIM = C_IN * KS * KS         # 72 = contraction dim of the conv matmul
CD = C_OUT * D_OUT            # 512 out-channels per group
ITERS = 3
P = 128
EPS = 1e-8
NPXT = 120                    # pixels per tile (4 output rows)
ROW_TILES = [(0, 4), (4, 4), (8, 4), (12, 4), (16, 4), (20, 4), (24, 4), (28, 2)]


def _tree_add(nc, tmp, pxs, axis, n, dst):
    """Reduce tmp[pxs] over `axis` (1=D, 2=c) by in-place halving adds;
    final level writes dst."""
    while n > 2:
        h = n // 2
        if axis == 1:
            nc.vector.tensor_add(tmp[pxs, 0:h], tmp[pxs, 0:h], tmp[pxs, h:n])
        else:
            nc.vector.tensor_add(tmp[pxs, :, 0:h], tmp[pxs, :, 0:h],
                                 tmp[pxs, :, h:n])
        n = h
    if axis == 1:
        nc.vector.tensor_add(dst, tmp[pxs, 0], tmp[pxs, 1])
    else:
        nc.vector.tensor_add(dst, tmp[pxs, :, 0], tmp[pxs, :, 1])


def _body(ctx, tc, x9h, wt, b0, out, zero_prior):
    nc = tc.nc
    consts = ctx.enter_context(tc.tile_pool(name="consts", bufs=1))
    x9pool = ctx.enter_context(tc.tile_pool(name="x9pool", bufs=2))
    upool = ctx.enter_context(tc.tile_pool(name="upool", bufs=2))
    spool = ctx.enter_context(tc.tile_pool(name="spool", bufs=2))
    sbpool = ctx.enter_context(tc.tile_pool(name="sbpool", bufs=2))
    rpool = ctx.enter_context(tc.tile_pool(name="rpool", bufs=1))
    psum_g = ctx.enter_context(tc.tile_pool(name="psum_g", bufs=4, space="PSUM"))
    psum_s = ctx.enter_context(tc.tile_pool(name="psum_s", bufs=2, space="PSUM"))

    w_sb = consts.tile([KDIM, D_IN, CD], BF16)
    nc.sync.dma_start(w_sb[:], wt)
    b0_sb = None
    if not zero_prior:
        b0_sb = consts.tile([P, D_IN, D_OUT], F32)
        nc.sync.dma_start(b0_sb[:], b0)

    for (r0, nr) in ROW_TILES:
        npx = nr * WO
        px0 = r0 * WO
        pxs = slice(0, npx)

        # ---- x9 tile DMA (host-side im2col), bf16
        x9 = x9pool.tile([KDIM, D_IN, NPXT], BF16, tag="x9")
        nc.sync.dma_start(x9[:, :, 0:npx], x9h[:, :, px0:px0 + npx])

        # ---- grouped conv; plus accumulating chain for sum_D u (zero prior)
        u_t = upool.tile([P, D_IN, C_OUT, D_OUT], BF16, tag="u")
        s_bf = sbpool.tile([P, C_OUT, D_OUT], BF16, tag="sbf")
        if zero_prior:
            pu_s = psum_s.tile([P, CD], F32, tag="pus")
        for g in range(D_IN):
            pu = psum_g.tile([P, CD], F32, tag="pu")
            nc.tensor.matmul(pu[pxs, :], x9[:, g, pxs], w_sb[:, g, :],
                             start=True, stop=True)
            if zero_prior:
                nc.tensor.matmul(pu_s[pxs, :], x9[:, g, pxs], w_sb[:, g, :],
                                 start=(g == 0), stop=(g == D_IN - 1))
            nc.scalar.copy(u_t[pxs, g], pu[pxs, :])

        # ---- routing state
        b_t = rpool.tile([P, D_IN, D_OUT], F32, tag="b")
        c_t = rpool.tile([P, D_IN, D_OUT], BF16, tag="c")
        e_t = rpool.tile([P, D_IN, D_OUT], BF16, tag="e")
        a_t = rpool.tile([P, D_IN, D_OUT], BF16, tag="a")
        tmp = rpool.tile([P, D_IN, C_OUT, D_OUT], BF16, tag="tmp")
        sq_t = rpool.tile([P, C_OUT, D_OUT], BF16, tag="sq")
        v_t = rpool.tile([P, C_OUT, D_OUT], BF16, tag="v")
        n2_t = rpool.tile([P, D_OUT], F32, tag="n2")
        lnt = rpool.tile([P, D_OUT], F32, tag="lnt")
        rs_t = rpool.tile([P, D_OUT], F32, tag="rs")
        n2p = rpool.tile([P, D_OUT], F32, tag="n2p")
        r_t = rpool.tile([P, D_OUT], F32, tag="r")
        f_t = rpool.tile([P, D_OUT], F32, tag="f")
        fb_t = rpool.tile([P, D_OUT], BF16, tag="fb")
        z_t = rpool.tile([P, D_IN], F32, tag="z")
        lz_t = rpool.tile([P, D_IN], F32, tag="lz")
        zr_t = rpool.tile([P, D_IN], BF16, tag="zr")
        s_t = spool.tile([P, C_OUT, D_OUT], F32, tag="s")

        if zero_prior:
            nc.scalar.activation(s_bf[pxs], pu_s[pxs, :], AF.Copy,
                                 scale=1.0 / D_IN)
        else:
            nc.scalar.copy(b_t[pxs], b0_sb[pxs])

        for it in range(ITERS):
            first = it == 0
            last = it == ITERS - 1

            # softmax over d -> c (skipped for uniform first iter)
            if not (first and zero_prior):
                nc.scalar.activation(e_t[pxs], b_t[pxs], AF.Exp)
                nc.vector.reduce_sum(z_t[pxs], e_t[pxs],
                                     axis=mybir.AxisListType.X)
                nc.scalar.activation(lz_t[pxs], z_t[pxs], AF.Ln)
                nc.scalar.activation(zr_t[pxs], lz_t[pxs], AF.Exp, scale=-1.0)
                nc.vector.tensor_mul(
                    c_t[pxs], e_t[pxs],
                    zr_t[pxs].unsqueeze(1).broadcast_to((npx, D_IN, D_OUT)))

                # s[c,d] = sum_D c[D,d] * u[D,c,d]
                nc.vector.tensor_mul(
                    tmp[pxs], u_t[pxs],
                    c_t[pxs].unsqueeze(2)
                    .broadcast_to((npx, D_IN, C_OUT, D_OUT)))
                dst = s_t[pxs] if last else s_bf[pxs]
                _tree_add(nc, tmp, pxs, 1, D_IN, dst)

            if last:
                break

            # squash over c: v = s * n2 / ((1+n2) * sqrt(n2+eps))
            nc.scalar.activation(sq_t[pxs], s_bf[pxs], AF.Square)
            nc.vector.reduce_sum(n2_t[pxs], sq_t[pxs].transpose([0, 2, 1]),
                                 axis=mybir.AxisListType.X)
            nc.vector.tensor_scalar_add(r_t[pxs], n2_t[pxs], EPS)
            nc.scalar.activation(lnt[pxs], r_t[pxs], AF.Ln)
            nc.scalar.activation(rs_t[pxs], lnt[pxs], AF.Exp, scale=-0.5)
            nc.scalar.add(n2p[pxs], n2_t[pxs], 1.0)
            nc.vector.reciprocal(r_t[pxs], n2p[pxs])
            nc.vector.tensor_mul(f_t[pxs], n2_t[pxs], r_t[pxs])
            nc.vector.tensor_mul(fb_t[pxs], f_t[pxs], rs_t[pxs])
            nc.vector.tensor_mul(
                v_t[pxs], s_bf[pxs],
                fb_t[pxs].unsqueeze(1).broadcast_to((npx, C_OUT, D_OUT)))

            # b[D,d] (+)= sum_c u[D,c,d] * v[c,d]
            nc.vector.tensor_mul(
                tmp[pxs], u_t[pxs],
                v_t[pxs].unsqueeze(1).broadcast_to((npx, D_IN, C_OUT, D_OUT)))
            if first and zero_prior:
                _tree_add(nc, tmp, pxs, 2, C_OUT, b_t[pxs])
            else:
                _tree_add(nc, tmp, pxs, 2, C_OUT, a_t[pxs])
                nc.vector.tensor_add(b_t[pxs], b_t[pxs], a_t[pxs])

        # ---- write s out as [px, (c,d)]; host transposes
        nc.sync.dma_start(out[px0:px0 + npx, :], s_t[pxs])


_CACHE = {}


def _build(zero_prior: bool):
    key = ("v4", zero_prior)
    if key in _CACHE:
        return _CACHE[key]
    nc = bacc.Bacc("TRN2", target_bir_lowering=False, debug=False,
                   enable_asserts=True, num_devices=B)
    x9h = nc.dram_tensor("x9h", [KDIM, D_IN, NPX], BF16,
                         kind="ExternalInput").ap()
    wt = nc.dram_tensor("wt", [KDIM, D_IN, CD], BF16,
                        kind="ExternalInput").ap()
    b0 = nc.dram_tensor("b0", [P, D_IN, D_OUT], F32,
                        kind="ExternalInput").ap()
    out = nc.dram_tensor("out", [NPX, CD], F32, kind="ExternalOutput").ap()
    with tile.TileContext(nc) as tc:
        with ExitStack() as ctx:
            _body(ctx, tc, x9h, wt, b0, out, zero_prior)
    nc.compile()
    _CACHE[key] = nc
    return nc


def _prep_inputs(x, conv_w, prior):
    bf16 = ml_dtypes.bfloat16
    # weights: rows (D,c,d) x (C,kh,kw) -> [k=(kh,kw,C), D, (c,d)]
    wt = conv_w.reshape(D_IN, C_OUT, D_OUT, C_IN, KS, KS)
    wt = np.ascontiguousarray(wt.transpose(4, 5, 3, 0, 1, 2)).reshape(
        KDIM, D_IN, CD).astype(bf16)
    pb = np.broadcast_to(prior.reshape(D_IN, D_OUT), (P, D_IN, D_OUT))
    b0 = np.ascontiguousarray(pb).astype(np.float32)
    # host-side im2col: x9[b, (kh,kw,C), D, px]
    x9 = np.empty((B, KDIM, D_IN, NPX), dtype=bf16)
    for kh in range(KS):
        for kw in range(KS):
            kk = (kh * KS + kw) * C_IN
            win = x[:, :, :, kh:kh + HO, kw:kw + WO].reshape(
                B, C_IN, D_IN, NPX)
            x9[:, kk:kk + C_IN] = win.astype(bf16)
    in_maps = [
        {"x9h": np.ascontiguousarray(x9[b]), "wt": wt, "b0": b0}
        for b in range(B)
    ]
    return in_maps


def _unpack_out(arr):
    return np.ascontiguousarray(
        arr.reshape(HO, WO, C_OUT, D_OUT).transpose(2, 3, 0, 1))


def kernel(x, conv_w, prior):
    x = np.asarray(x, dtype=np.float32)
    conv_w = np.asarray(conv_w, dtype=np.float32)
    prior = np.asarray(prior, dtype=np.float32)
    zero_prior = not np.any(prior)
    nc = _build(zero_prior)
    in_maps = _prep_inputs(x, conv_w, prior)
    res = run_bass_kernel_spmd(nc, in_maps, list(range(B)))
    outs = [_unpack_out(res.results[b]["out"]) for b in range(B)]
    return np.stack(outs, axis=0).astype(np.float32)


# revision 5
# speedup vs baseline: 2.8272x; 1.0609x over previous
"""ConvCaps dynamic-routing kernel for 8 TRN2 NeuronCores.

Strategy (data-parallel over batch B=8, one batch element per core):
  - im2col done on HOST -> x9h [72, D, 900] bf16; one DMA per pixel tile.
  - Grouped 3x3 conv as one bf16 matmul per group per pixel tile:
    stationary = im2col patches [72, npx], moving = weights [72, 512],
    PSUM out [npx, 512] -> u tile in SBUF [px, D, c, d] bf16 (Scalar copy).
  - A second, PSUM-accumulating matmul chain over the 32 groups yields
    sum_D u for free -> iteration-1 s under a zero prior (uniform c).
  - Routing runs on the Vector engine in bf16 (2x packed mode):
    multiplies with 0-stride broadcast APs; reductions as contiguous
    halving add-trees (bf16 tensor_add at 2x beats 1x tensor_reduce).
  - All Scalar activations stay in ONE table set
    (natural_log_exp_and_others: Exp/Ln/Square/Copy); 1/sqrt(n2+eps)
    is computed as Exp(-0.5*Ln(n2+eps)) to avoid sqrt-set thrash.
  - Output s is DMA'd out as [px, (c,d)]; host transposes.
"""

import numpy as np
from contextlib import ExitStack

import ml_dtypes

import concourse.bacc as bacc
import concourse.tile as tile
import concourse.mybir as mybir
from concourse.bass_utils import run_bass_kernel_spmd

F32 = mybir.dt.float32
F16 = mybir.dt.float16
AF = mybir.ActivationFunctionType

B = 8
C_IN, D_IN = 8, 32
C_OUT, D_OUT = 16, 32
KS = 3
H = W = 32
HO = WO = 30
NPX = HO * WO                 # 900 output pixels per batch element
KDIM = C_IN * KS * KS         # 72 = contraction dim of the conv matmul
CD = C_OUT * D_OUT            # 512 out-channels per group
ITERS = 3
P = 128
EPS = 1e-8
NPXT = 120                    # pixels per tile (4 output rows)
ROW_TILES = [(0, 4), (4, 4), (8, 4), (12, 4), (16, 4), (20, 4), (24, 4), (28, 2)]


def _tree_add(nc, tmp, pxs, axis, n, dst):
    """Reduce tmp[pxs] over `axis` (1=D, 2=c) by in-place halving adds;
    final level writes dst."""
    while n > 2:
        h = n // 2
        if axis == 1:
            nc.vector.tensor_add(tmp[pxs, 0:h], tmp[pxs, 0:h], tmp[pxs, h:n])
        else:
            nc.vector.tensor_add(tmp[pxs, :, 0:h], tmp[pxs, :, 0:h],
                                 tmp[pxs, :, h:n])
        n = h
    if axis == 1:
        nc.vector.tensor_add(dst, tmp[pxs, 0], tmp[pxs, 1])
    else:
        nc.vector.tensor_add(dst, tmp[pxs, :, 0], tmp[pxs, :, 1])


def _body(ctx, tc, x9h, wt, b0, out, zero_prior):
    nc = tc.nc
    consts = ctx.enter_context(tc.tile_pool(name="consts", bufs=1))
    x9pool = ctx.enter_context(tc.tile_pool(name="x9pool", bufs=2))
    upool = ctx.enter_context(tc.tile_pool(name="upool", bufs=2))
    spool = ctx.enter_context(tc.tile_pool(name="spool", bufs=2))
    sbpool = ctx.enter_context(tc.tile_pool(name="sbpool", bufs=2))
    rpool = ctx.enter_context(tc.tile_pool(name="rpool", bufs=1))
    psum_g = ctx.enter_context(tc.tile_pool(name="psum_g", bufs=4, space="PSUM"))
    psum_s = ctx.enter_context(tc.tile_pool(name="psum_s", bufs=2, space="PSUM"))

    w_sb = consts.tile([KDIM, D_IN, CD], F16)
    nc.sync.dma_start(w_sb[:], wt)
    b0_sb = None
    if not zero_prior:
        b0_sb = consts.tile([P, D_IN, D_OUT], F32)
        nc.sync.dma_start(b0_sb[:], b0)

    for (r0, nr) in ROW_TILES:
        npx = nr * WO
        px0 = r0 * WO
        pxs = slice(0, npx)

        # ---- x9 tile DMA (host-side im2col), bf16
        x9 = x9pool.tile([KDIM, D_IN, NPXT], F16, tag="x9")
        nc.sync.dma_start(x9[:, :, 0:npx], x9h[:, :, px0:px0 + npx])

        # ---- grouped conv; plus accumulating chain for sum_D u (zero prior)
        u_t = upool.tile([P, D_IN, C_OUT, D_OUT], F16, tag="u")
        s_bf = sbpool.tile([P, C_OUT, D_OUT], F16, tag="sbf")
        if zero_prior:
            pu_s = psum_s.tile([P, CD], F32, tag="pus")
        for g in range(D_IN):
            pu = psum_g.tile([P, CD], F32, tag="pu")
            nc.tensor.matmul(pu[pxs, :], x9[:, g, pxs], w_sb[:, g, :],
                             start=True, stop=True)
            if zero_prior:
                nc.tensor.matmul(pu_s[pxs, :], x9[:, g, pxs], w_sb[:, g, :],
                                 start=(g == 0), stop=(g == D_IN - 1))
            nc.scalar.copy(u_t[pxs, g], pu[pxs, :])

        # ---- routing state
        b_t = rpool.tile([P, D_IN, D_OUT], F32, tag="b")
        c_t = rpool.tile([P, D_IN, D_OUT], F16, tag="c")
        e_t = rpool.tile([P, D_IN, D_OUT], F16, tag="e")
        a_t = rpool.tile([P, D_IN, D_OUT], F16, tag="a")
        tmp = rpool.tile([P, D_IN, C_OUT, D_OUT], F16, tag="tmp")
        sq_t = rpool.tile([P, C_OUT, D_OUT], F16, tag="sq")
        v_t = rpool.tile([P, C_OUT, D_OUT], F16, tag="v")
        n2_t = rpool.tile([P, D_OUT], F32, tag="n2")
        lnt = rpool.tile([P, D_OUT], F32, tag="lnt")
        rs_t = rpool.tile([P, D_OUT], F32, tag="rs")
        n2p = rpool.tile([P, D_OUT], F32, tag="n2p")
        r_t = rpool.tile([P, D_OUT], F32, tag="r")
        f_t = rpool.tile([P, D_OUT], F32, tag="f")
        fb_t = rpool.tile([P, D_OUT], F16, tag="fb")
        z_t = rpool.tile([P, D_IN], F32, tag="z")
        lz_t = rpool.tile([P, D_IN], F32, tag="lz")
        zr_t = rpool.tile([P, D_IN], F16, tag="zr")
        s_t = spool.tile([P, C_OUT, D_OUT], F32, tag="s")

        if zero_prior:
            nc.scalar.activation(s_bf[pxs], pu_s[pxs, :], AF.Copy,
                                 scale=1.0 / D_IN)
        else:
            nc.scalar.copy(b_t[pxs], b0_sb[pxs])

        for it in range(ITERS):
            first = it == 0
            last = it == ITERS - 1

            # softmax over d -> c (skipped for uniform first iter)
            if not (first and zero_prior):
                nc.scalar.activation(e_t[pxs], b_t[pxs], AF.Exp)
                nc.vector.reduce_sum(z_t[pxs], e_t[pxs],
                                     axis=mybir.AxisListType.X)
                nc.scalar.activation(lz_t[pxs], z_t[pxs], AF.Ln)
                nc.scalar.activation(zr_t[pxs], lz_t[pxs], AF.Exp, scale=-1.0)
                nc.vector.tensor_mul(
                    c_t[pxs], e_t[pxs],
                    zr_t[pxs].unsqueeze(1).broadcast_to((npx, D_IN, D_OUT)))

                # s[c,d] = sum_D c[D,d] * u[D,c,d]
                nc.vector.tensor_mul(
                    tmp[pxs], u_t[pxs],
                    c_t[pxs].unsqueeze(2)
                    .broadcast_to((npx, D_IN, C_OUT, D_OUT)))
                dst = s_t[pxs] if last else s_bf[pxs]
                _tree_add(nc, tmp, pxs, 1, D_IN, dst)

            if last:
                break

            # squash over c: v = s * n2 / ((1+n2) * sqrt(n2+eps))
            nc.scalar.activation(sq_t[pxs], s_bf[pxs], AF.Square)
            nc.vector.reduce_sum(n2_t[pxs], sq_t[pxs].transpose([0, 2, 1]),
                                 axis=mybir.AxisListType.X)
            nc.vector.tensor_scalar_add(r_t[pxs], n2_t[pxs], EPS)
            nc.scalar.activation(lnt[pxs], r_t[pxs], AF.Ln)
            nc.scalar.activation(rs_t[pxs], lnt[pxs], AF.Exp, scale=-0.5)
            nc.scalar.add(n2p[pxs], n2_t[pxs], 1.0)
            nc.vector.reciprocal(r_t[pxs], n2p[pxs])
            nc.vector.tensor_mul(f_t[pxs], n2_t[pxs], r_t[pxs])
            nc.vector.tensor_mul(fb_t[pxs], f_t[pxs], rs_t[pxs])
            nc.vector.tensor_mul(
                v_t[pxs], s_bf[pxs],
                fb_t[pxs].unsqueeze(1).broadcast_to((npx, C_OUT, D_OUT)))

            # b[D,d] (+)= sum_c u[D,c,d] * v[c,d]
            nc.vector.tensor_mul(
                tmp[pxs], u_t[pxs],
                v_t[pxs].unsqueeze(1).broadcast_to((npx, D_IN, C_OUT, D_OUT)))
            if first and zero_prior:
                _tree_add(nc, tmp, pxs, 2, C_OUT, b_t[pxs])
            else:
                _tree_add(nc, tmp, pxs, 2, C_OUT, a_t[pxs])
                nc.vector.tensor_add(b_t[pxs], b_t[pxs], a_t[pxs])

        # ---- write s out as [px, (c,d)]; host transposes
        nc.sync.dma_start(out[px0:px0 + npx, :], s_t[pxs])


_CACHE = {}


def _pin_act_tables(arch):
    """Make natural_log_exp_and_others the only set advertising Exp/Ln so
    the act-table-load pass stops alternating between exp_and_others and
    natural_log (a ~2.7us table DMA per switch, 8x per pixel tile). The
    pinned set genuinely contains Exp/Ln/Square/Copy/Identity; set ids
    keep their act_info.json positions, so the loads stay correct."""
    import concourse.hw_specs as hw_specs
    tables = hw_specs.get_activation_tables(arch)  # functools.cache'd dict
    keep = "natural_log_exp_and_others"
    assert keep in tables
    for fn in (AF.Exp, AF.Ln):
        assert fn in tables[keep]
        for name, fns in tables.items():
            if name != keep:
                fns.discard(fn)


def _build(zero_prior: bool):
    key = ("v5", zero_prior)
    if key in _CACHE:
        return _CACHE[key]
    nc = bacc.Bacc("TRN2", target_bir_lowering=False, debug=False,
                   enable_asserts=True, num_devices=B)
    _pin_act_tables(nc.m.arch)
    x9h = nc.dram_tensor("x9h", [KDIM, D_IN, NPX], F16,
                         kind="ExternalInput").ap()
    wt = nc.dram_tensor("wt", [KDIM, D_IN, CD], F16,
                        kind="ExternalInput").ap()
    b0 = nc.dram_tensor("b0", [P, D_IN, D_OUT], F32,
                        kind="ExternalInput").ap()
    out = nc.dram_tensor("out", [NPX, CD], F32, kind="ExternalOutput").ap()
    with tile.TileContext(nc) as tc:
        with ExitStack() as ctx:
            _body(ctx, tc, x9h, wt, b0, out, zero_prior)
    nc.compile()
    _CACHE[key] = nc
    return nc


def _prep_inputs(x, conv_w, prior):
    f16 = np.float16
    # weights: rows (D,c,d) x (C,kh,kw) -> [k=(kh,kw,C), D, (c,d)]
    wt = conv_w.reshape(D_IN, C_OUT, D_OUT, C_IN, KS, KS)
    wt = np.ascontiguousarray(wt.transpose(4, 5, 3, 0, 1, 2)).reshape(
        KDIM, D_IN, CD).astype(f16)
    pb = np.broadcast_to(prior.reshape(D_IN, D_OUT), (P, D_IN, D_OUT))
    b0 = np.ascontiguousarray(pb).astype(np.float32)
    # host-side im2col: x9[b, (kh,kw,C), D, px]
    x9 = np.empty((B, KDIM, D_IN, NPX), dtype=f16)
    for kh in range(KS):
        for kw in range(KS):
            kk = (kh * KS + kw) * C_IN
            win = x[:, :, :, kh:kh + HO, kw:kw + WO].reshape(
                B, C_IN, D_IN, NPX)
            x9[:, kk:kk + C_IN] = win.astype(f16)
    in_maps = [
        {"x9h": np.ascontiguousarray(x9[b]), "wt": wt, "b0": b0}
        for b in range(B)
    ]
    return in_maps


def _unpack_out(arr):
    return np.ascontiguousarray(
        arr.reshape(HO, WO, C_OUT, D_OUT).transpose(2, 3, 0, 1))


def kernel(x, conv_w, prior):
    x = np.asarray(x, dtype=np.float32)
    conv_w = np.asarray(conv_w, dtype=np.float32)
    prior = np.asarray(prior, dtype=np.float32)
    zero_prior = not np.any(prior)
    nc = _build(zero_prior)
    in_maps = _prep_inputs(x, conv_w, prior)
    res = run_bass_kernel_spmd(nc, in_maps, list(range(B)))
    outs = [_unpack_out(res.results[b]["out"]) for b in range(B)]
    return np.stack(outs, axis=0).astype(np.float32)


# revision 8
# speedup vs baseline: 2.9514x; 1.0439x over previous
"""ConvCaps dynamic-routing kernel for 8 TRN2 NeuronCores.

Strategy (data-parallel over batch B=8, one batch element per core):
  - im2col done on HOST -> x9h [72, D, 900] fp16; one DMA per pixel tile.
  - Grouped 3x3 conv as one fp16 matmul per group per pixel tile:
    stationary = im2col patches [72, npx], moving = weights [72, 512],
    PSUM out [npx, 512] -> u tile in SBUF [px, D, c, d] fp16 (Scalar copy).
  - A second, PSUM-accumulating matmul chain over the 32 groups yields
    sum_D u for free -> iteration-1 s under a zero prior (uniform c).
  - Routing runs on the Vector engine in fp16 (2x packed mode):
    multiplies with 0-stride broadcast APs; reductions as contiguous
    in-place halving add-trees (fp16 tensor_add at 2x beats 1x
    tensor_reduce). Softmax normalization is applied post-reduction
    (s = (sum_D e*u) * (1/Z)), keeping the Ln/Exp of Z off the DVE
    critical path.
  - Pixel tiles are processed in PAIRS with their routing phases
    interleaved, so one tile's DVE work hides the other's Scalar
    latencies (exp/ln handoffs).
  - All Scalar activations stay in ONE table set
    (natural_log_exp_and_others: Exp/Ln/Square/Copy/Identity);
    1/sqrt(n2+eps) is Exp(-0.5*Ln(n2+eps)) to avoid sqrt-set thrash,
    and _pin_act_tables stops the allocator from alternating sets.
  - Output s is DMA'd out as [px, (c,d)]; host transposes.
"""

import numpy as np
from contextlib import ExitStack

import concourse.bacc as bacc
import concourse.tile as tile
import concourse.mybir as mybir
from concourse.bass_utils import run_bass_kernel_spmd

F32 = mybir.dt.float32
F16 = mybir.dt.float16
AF = mybir.ActivationFunctionType

B = 8
C_IN, D_IN = 8, 32
C_OUT, D_OUT = 16, 32
KS = 3
H = W = 32
HO = WO = 30
NPX = HO * WO                 # 900 output pixels per batch element
KDIM = C_IN * KS * KS         # 72 = contraction dim of the conv matmul
CD = C_OUT * D_OUT            # 512 out-channels per group
ITERS = 3
P = 128
EPS = 1e-8
NPXT = 120                    # pixels per tile (4 output rows)
ROW_TILES = [(0, 4), (4, 4), (8, 4), (12, 4), (16, 4), (20, 4), (24, 4), (28, 2)]


def _tree_add(nc, t, pxs, axis, n, dst):
    """Reduce t[pxs] over `axis` (1 or 2) by in-place halving adds; the
    final level writes dst."""
    while n > 2:
        h = n // 2
        if axis == 1:
            nc.vector.tensor_add(t[pxs, 0:h], t[pxs, 0:h], t[pxs, h:n])
        else:
            nc.vector.tensor_add(t[pxs, :, 0:h], t[pxs, :, 0:h], t[pxs, :, h:n])
        n = h
    if axis == 1:
        nc.vector.tensor_add(dst, t[pxs, 0], t[pxs, 1])
    else:
        nc.vector.tensor_add(dst, t[pxs, :, 0], t[pxs, :, 1])


def _conv_tile(nc, pools, st, x9h, w_sb, b0_sb, zero_prior):
    """DMA + grouped conv for one pixel tile; fills st with tiles."""
    npx, px0, pxs = st["npx"], st["px0"], st["pxs"]
    x9 = pools["x9"].tile([KDIM, D_IN, NPXT], F16, tag="x9")
    nc.sync.dma_start(x9[:, :, 0:npx], x9h[:, :, px0:px0 + npx])

    u_t = pools["u"].tile([P, D_IN, C_OUT, D_OUT], F16, tag="u")
    s_bf = pools["r2"].tile([P, C_OUT, D_OUT], F16, tag="sbf")
    if zero_prior:
        pu_s = pools["ps"].tile([P, CD], F32, tag="pus")
    for g in range(D_IN):
        pu = pools["pg"].tile([P, CD], F32, tag="pu")
        nc.tensor.matmul(pu[pxs, :], x9[:, g, pxs], w_sb[:, g, :],
                         start=True, stop=True)
        if zero_prior:
            nc.tensor.matmul(pu_s[pxs, :], x9[:, g, pxs], w_sb[:, g, :],
                             start=(g == 0), stop=(g == D_IN - 1))
        nc.scalar.copy(u_t[pxs, g], pu[pxs, :])

    r2 = pools["r2"]
    st["u"] = u_t
    st["sbf"] = s_bf
    st["b"] = r2.tile([P, D_IN, D_OUT], F32, tag="b", name="b_t")
    st["e"] = r2.tile([P, D_IN, D_OUT], F16, tag="e", name="e_t")
    st["a"] = r2.tile([P, D_IN, D_OUT], F16, tag="a", name="a_t")
    st["sq"] = r2.tile([P, C_OUT, D_OUT], F16, tag="sq", name="sq_t")
    st["v"] = r2.tile([P, C_OUT, D_OUT], F16, tag="v", name="v_t")
    st["su"] = r2.tile([P, C_OUT, D_OUT], F16, tag="su", name="su_t")
    st["s"] = pools["s"].tile([P, C_OUT, D_OUT], F32, tag="s", name="s_t")
    for name in ("n2", "te", "lnt", "rs", "n2p", "rcp", "f0"):
        st[name] = r2.tile([P, D_OUT], F32, tag=name, name=name + "_t")
    st["fb"] = r2.tile([P, D_OUT], F16, tag="fb", name="fb_t")
    st["z"] = r2.tile([P, D_IN], F32, tag="z", name="z_t")
    st["lz"] = r2.tile([P, D_IN], F32, tag="lz", name="lz_t")
    st["zr"] = r2.tile([P, D_IN], F16, tag="zr", name="zr_t")

    if zero_prior:
        nc.scalar.activation(s_bf[pxs], pu_s[pxs, :], AF.Copy,
                             scale=1.0 / D_IN)
    else:
        nc.scalar.copy(st["b"][pxs], b0_sb[pxs])


def _phases(nc, st, tmp, zero_prior):
    """Return the routing phase closures for one tile, in order."""
    npx, pxs = st["npx"], st["pxs"]

    def softmax():
        nc.scalar.activation(st["e"][pxs], st["b"][pxs], AF.Exp)
        nc.vector.reduce_sum(st["z"][pxs], st["e"][pxs],
                             axis=mybir.AxisListType.X)
        nc.scalar.activation(st["lz"][pxs], st["z"][pxs], AF.Ln)
        nc.scalar.activation(st["zr"][pxs], st["lz"][pxs], AF.Exp, scale=-1.0)

    def spass(last):
        # s[c,d] = (sum_D e[D,d] * u[D,c,d]) / Z[d]
        nc.vector.tensor_mul(
            tmp[pxs], st["u"][pxs],
            st["e"][pxs].unsqueeze(2).broadcast_to((npx, D_IN, C_OUT, D_OUT)))
        _tree_add(nc, tmp, pxs, 1, D_IN, st["su"][pxs])
        dst = st["s"] if last else st["sbf"]
        nc.vector.tensor_mul(
            dst[pxs], st["su"][pxs],
            st["zr"][pxs].unsqueeze(1).broadcast_to((npx, C_OUT, D_OUT)))

    def squash():
        # v = s * n2 / ((1+n2) * sqrt(n2+eps)); rsqrt via Exp(-0.5*Ln)
        nc.scalar.activation(st["sq"][pxs], st["sbf"][pxs], AF.Square)
        _tree_add(nc, st["sq"], pxs, 1, C_OUT, st["n2"][pxs])
        nc.vector.tensor_scalar_add(st["te"][pxs], st["n2"][pxs], EPS)
        nc.scalar.activation(st["lnt"][pxs], st["te"][pxs], AF.Ln)
        nc.scalar.activation(st["rs"][pxs], st["lnt"][pxs], AF.Exp,
                             scale=-0.5)
        nc.scalar.add(st["n2p"][pxs], st["n2"][pxs], 1.0)
        nc.vector.reciprocal(st["rcp"][pxs], st["n2p"][pxs])
        nc.vector.tensor_mul(st["f0"][pxs], st["n2"][pxs], st["rcp"][pxs])
        nc.vector.tensor_mul(st["fb"][pxs], st["f0"][pxs], st["rs"][pxs])
        nc.vector.tensor_mul(
            st["v"][pxs], st["sbf"][pxs],
            st["fb"][pxs].unsqueeze(1).broadcast_to((npx, C_OUT, D_OUT)))

    def apass(first):
        # b[D,d] (+)= sum_c u[D,c,d] * v[c,d]
        nc.vector.tensor_mul(
            tmp[pxs], st["u"][pxs],
            st["v"][pxs].unsqueeze(1).broadcast_to((npx, D_IN, C_OUT, D_OUT)))
        if first and zero_prior:
            _tree_add(nc, tmp, pxs, 2, C_OUT, st["b"][pxs])
        else:
            _tree_add(nc, tmp, pxs, 2, C_OUT, st["a"][pxs])
            nc.vector.tensor_add(st["b"][pxs], st["b"][pxs], st["a"][pxs])

    ph = []
    for it in range(ITERS):
        first, last = it == 0, it == ITERS - 1
        if not (first and zero_prior):
            ph.append(softmax)
            ph.append(lambda last=last: spass(last))
        if last:
            break
        ph.append(squash)
        ph.append(lambda first=first: apass(first))
    return ph


def _body(ctx, tc, x9h, wt, b0, out, zero_prior):
    nc = tc.nc
    pools = {
        "c": ctx.enter_context(tc.tile_pool(name="consts", bufs=1)),
        "x9": ctx.enter_context(tc.tile_pool(name="x9pool", bufs=2)),
        "u": ctx.enter_context(tc.tile_pool(name="upool", bufs=2)),
        "s": ctx.enter_context(tc.tile_pool(name="spool", bufs=2)),
        "r2": ctx.enter_context(tc.tile_pool(name="rpool", bufs=2)),
        "tp": ctx.enter_context(tc.tile_pool(name="tmppool", bufs=1)),
        "pg": ctx.enter_context(tc.tile_pool(name="psum_g", bufs=4,
                                             space="PSUM")),
        "ps": ctx.enter_context(tc.tile_pool(name="psum_s", bufs=2,
                                             space="PSUM")),
    }
    w_sb = pools["c"].tile([KDIM, D_IN, CD], F16)
    nc.sync.dma_start(w_sb[:], wt)
    b0_sb = None
    if not zero_prior:
        b0_sb = pools["c"].tile([P, D_IN, D_OUT], F32)
        nc.sync.dma_start(b0_sb[:], b0)

    for pi in range(0, len(ROW_TILES), 2):
        pair = ROW_TILES[pi:pi + 2]
        states = []
        for (r0, nr) in pair:
            st = {"npx": nr * WO, "px0": r0 * WO, "pxs": slice(0, nr * WO)}
            _conv_tile(nc, pools, st, x9h, w_sb, b0_sb, zero_prior)
            states.append(st)
        # one DVE-only scratch shared by the pair (the in-order DVE queue
        # serializes its users; sharing saves 32KB of SBUF)
        tmp = pools["tp"].tile([P, D_IN, C_OUT, D_OUT], F16, tag="tmp")
        phases = [_phases(nc, st, tmp, zero_prior) for st in states]
        for group in zip(*phases):
            for ph in group:
                ph()
        for st in states:
            nc.sync.dma_start(out[st["px0"]:st["px0"] + st["npx"], :],
                              st["s"][st["pxs"]])


_CACHE = {}


def _pin_act_tables(arch):
    """Make natural_log_exp_and_others the only set advertising Exp/Ln so
    the act-table-load pass stops alternating between exp_and_others and
    natural_log (a ~2.7us table DMA per switch, 8x per pixel tile). The
    pinned set genuinely contains Exp/Ln/Square/Copy/Identity; set ids
    keep their act_info.json positions, so the loads stay correct."""
    import concourse.hw_specs as hw_specs
    tables = hw_specs.get_activation_tables(arch)  # functools.cache'd dict
    keep = "natural_log_exp_and_others"
    assert keep in tables
    for fn in (AF.Exp, AF.Ln):
        assert fn in tables[keep]
        for name, fns in tables.items():
            if name != keep:
                fns.discard(fn)


def _build(zero_prior: bool):
    key = ("v6", zero_prior)
    if key in _CACHE:
        return _CACHE[key]
    nc = bacc.Bacc("TRN2", target_bir_lowering=False, debug=False,
                   enable_asserts=True, num_devices=B)
    _pin_act_tables(nc.m.arch)
    x9h = nc.dram_tensor("x9h", [KDIM, D_IN, NPX], F16,
                         kind="ExternalInput").ap()
    wt = nc.dram_tensor("wt", [KDIM, D_IN, CD], F16,
                        kind="ExternalInput").ap()
    b0 = nc.dram_tensor("b0", [P, D_IN, D_OUT], F32,
                        kind="ExternalInput").ap()
    out = nc.dram_tensor("out", [NPX, CD], F32, kind="ExternalOutput").ap()
    with tile.TileContext(nc) as tc:
        with ExitStack() as ctx:
            _body(ctx, tc, x9h, wt, b0, out, zero_prior)
    nc.compile()
    _CACHE[key] = nc
    return nc


def _prep_inputs(x, conv_w, prior):
    f16 = np.float16
    # weights: rows (D,c,d) x (C,kh,kw) -> [k=(kh,kw,C), D, (c,d)]
    wt = conv_w.reshape(D_IN, C_OUT, D_OUT, C_IN, KS, KS)
    wt = np.ascontiguousarray(wt.transpose(4, 5, 3, 0, 1, 2)).reshape(
        KDIM, D_IN, CD).astype(f16)
    pb = np.broadcast_to(prior.reshape(D_IN, D_OUT), (P, D_IN, D_OUT))
    b0 = np.ascontiguousarray(pb).astype(np.float32)
    # host-side im2col: x9[b, (kh,kw,C), D, px]
    x9 = np.empty((B, KDIM, D_IN, NPX), dtype=f16)
    for kh in range(KS):
        for kw in range(KS):
            kk = (kh * KS + kw) * C_IN
            win = x[:, :, :, kh:kh + HO, kw:kw + WO].reshape(
                B, C_IN, D_IN, NPX)
            x9[:, kk:kk + C_IN] = win.astype(f16)
    in_maps = [
        {"x9h": np.ascontiguousarray(x9[b]), "wt": wt, "b0": b0}
        for b in range(B)
    ]
    return in_maps


def _unpack_out(arr):
    return np.ascontiguousarray(
        arr.reshape(HO, WO, C_OUT, D_OUT).transpose(2, 3, 0, 1))


def kernel(x, conv_w, prior):
    x = np.asarray(x, dtype=np.float32)
    conv_w = np.asarray(conv_w, dtype=np.float32)
    prior = np.asarray(prior, dtype=np.float32)
    zero_prior = not np.any(prior)
    nc = _build(zero_prior)
    in_maps = _prep_inputs(x, conv_w, prior)
    res = run_bass_kernel_spmd(nc, in_maps, list(range(B)))
    outs = [_unpack_out(res.results[b]["out"]) for b in range(B)]
    return np.stack(outs, axis=0).astype(np.float32)
